# revision 19
# baseline (speedup 1.0000x reference)
"""Trainium2 Bass kernel for nn_MemoryUpdate (gated LIF memory update).

Reference computation (fp32):
    k         = einsum('tbnd,od->tbno', kv, Wg)          # kv @ Wg^T
    gate_mean = mean_t'( k[t', b, nkv, d] )              # [Nkv, B, 1, D], Nkv == T
    update    = gate_mean[t, b, d] * q[t, b, n, d]       # broadcast over n
    spikes    = LIF over t: v' = (v + u)/2 ; s = v' >= 0.5 ; v = v' * (1 - s)

Shapes: q [4, 32, 1024, 512], kv [4, 32, 4, 512], Wg [512, 512] -> out [4, 32, 1024, 512].

Strategy: data-parallel over B across 8 cores (B_loc = 4), d-major on device.

The LIF recurrence is rescaled by 2^t so the 1/2 leak becomes exact integer
powers folded into per-step constants:  b_t := 2^t v'_t satisfies
    b_t = G_t (x) q_t + W_{t-1},   G_t = 2^{t-1} gate_mean_t   (host-folded)
    s_t = (b_t >= thr_t),          thr_t = 2^{t-1}
    W_t = (b_t < thr_t) * b_t
Host feeds q TRANSPOSED to [T, B, D, Nq] so d lands on SBUF partitions and the
gate G_t[d] becomes a per-partition scalar: the charge step is then ONE DVE
scalar_tensor_tensor (b = (q * g_ptr) + W) instead of a broadcast-multiply plus
add, and t=0 is a half-rate tensor_scalar. Per-core DVE drops ~174us -> ~116us.

Per core per (t, b): tile [128 part = d%128, free = (d//128, n)] (4 KiB
contiguous DRAM runs per (dc, n)).  Engine budget (model, per core):
  DVE  ~116us: charge stt x48 slices + t0 ts x16 + reset-mask stt x12
  ACT  ~62us:  threshold sigmoid (saturating, exact 0/1) x16 + 7 store issues
  SP   queue:  16 q loads (+kvT/wgT) ~97us
  ACT  queue:  7 stores ~41us;  Pool queue: 7 stores ~41us; 2 stores on SP
  PE:   tiny gate matmul gT[d, (b,t)] = sum_d' WgT[d', d] kvsum[d', (b,t)]
"""

import sys

for p in ("/opt/trn_rl_repo", "/root/.axon_site/_ro/trn_rl_repo"):
    if p not in sys.path:
        sys.path.insert(0, p)

import numpy as np

import concourse.bass as bass
import concourse.mybir as mybir
import concourse.tile as tile
from concourse import bacc
from concourse.bass_utils import run_bass_kernel_spmd

# Problem constants (hardcoded per harness contract)
T, B, NQ, NKV, D = 4, 32, 1024, 4, 512
N_CORES = 8
B_LOC = B // N_CORES  # 4
V_TH = 0.5
P = 128               # partitions
DC = D // P           # 4 d-chunks
FREE = DC * NQ        # 4096 free elements per tile

FP32 = mybir.dt.float32
BF16 = mybir.dt.bfloat16
Alu = mybir.AluOpType
UNROLL = 8  # static inner unroll inside the timing-mode For_i loop
_BIG = 1.0e30  # threshold-comparison scale; saturates sigmoid to exact 0/1
THR = [0.5 * (2.0 ** t) for t in range(T)]  # per-step threshold 2^(t-1)


def build_kernel(repeats=1, timing_mode=False, num_devices=N_CORES,
                 use_accum=False):
    # Bacc (not raw Bass): its compile() legalizes multi-sem waits, which the
    # walrus CoreV3 codegen can't carry on a single compute instruction.
    nc = bacc.Bacc("TRN2", target_bir_lowering=False, debug=False,
                   num_devices=num_devices)

    if timing_mode:
        # timing-only variant: big tensors live in internal DRAM so the wall
        # clock isn't dominated by host<->device transfers; the main body runs
        # `repeats` times in an on-device loop.
        q = nc.dram_tensor("q_int", [T, B_LOC, D, NQ], FP32).ap()
        out = nc.dram_tensor("out_int", [T, B_LOC, D, NQ], BF16).ap()
        dummy = nc.dram_tensor("tiny_out", [P, 16], FP32, kind="ExternalOutput").ap()
    else:
        q = nc.dram_tensor("q", [T, B_LOC, D, NQ], FP32, kind="ExternalInput").ap()
        # spikes are exactly 0.0/1.0 -> bf16 is lossless and halves the
        # store traffic; the host casts back to fp32
        out = nc.dram_tensor("out", [T, B_LOC, D, NQ], BF16, kind="ExternalOutput").ap()
        dummy = None
    kvT = nc.dram_tensor("kvT", [D, T * B_LOC * NKV], FP32, kind="ExternalInput").ap()
    wgT = nc.dram_tensor("wgT", [D, D], FP32, kind="ExternalInput").ap()

    # Host supplies q/out in partition-interleaved d-major layout
    # [t, b, p, dc, n] (row p*DC+dc holds d = dc*128+p), so each partition's
    # (dc, n) free block is 16 KiB contiguous in DRAM -> fat DMA descriptors.
    q_v = q.rearrange("t b (p dc) n -> t b p dc n", p=P, dc=DC)
    out_v = out.rearrange("t b (p dc) n -> t b p dc n", p=P, dc=DC)
    # kvT rows: d = c*128 + p ; cols: i = t'*16 + b*4 + nkv
    kvT_v = kvT.rearrange("(c p) i -> p c i", p=P)
    wgT_v = wgT.rearrange("(c p) o -> p c o", p=P)
    NI = T * B_LOC * NKV  # 64
    NG = B_LOC * NKV      # 16 gate columns (b*4 + t)

    with tile.TileContext(nc) as tc:
        with (
            tc.tile_pool(name="const", bufs=1) as const_pool,
            tc.tile_pool(name="qp", bufs=5) as q_pool,
            tc.tile_pool(name="wp", bufs=2) as w_pool,
            tc.tile_pool(name="sp", bufs=3) as s_pool,
            tc.tile_pool(name="psg", bufs=1, space="PSUM") as psg_pool,
        ):
            # per-partition bias vectors for the threshold sigmoid (one per t);
            # also feeds a dummy activation that pre-loads the sigmoid ACT
            # table so the first real threshold doesn't pay the 1.3us load.
            thr_bias = const_pool.tile([P, T], FP32, tag="thrb")
            for t in range(T):
                nc.vector.memset(thr_bias[:, t:t + 1], -THR[t] * _BIG)
            # ---- gate computation (kvT/wgT ride the ACT queue so the SP
            # queue starts streaming q immediately) ----
            kvT_sb = const_pool.tile([P, 4 * NI], FP32, tag="kvT")
            nc.scalar.dma_start(kvT_sb[:].rearrange("p (c i) -> p c i", c=4), kvT_v)
            wgT_sb = const_pool.tile([P, 4 * D], FP32, tag="wgT")
            nc.scalar.dma_start(wgT_sb[:].rearrange("p (c o) -> p c o", c=4), wgT_v)
            # dummy activation pre-loads the sigmoid ACT table (~1.3us)
            warm = const_pool.tile([P, 1], FP32, tag="warm")
            nc.scalar.activation(
                warm[:], thr_bias[:, 0:1],
                mybir.ActivationFunctionType.Sigmoid, bias=0.0, scale=0.0,
            )

            # sum over t' of kvT (free layout per chunk: i = t'*16 + (b*4+nkv));
            # the 2^(t-1)/T gate scaling is folded into kvT host-side.
            kv4 = kvT_sb[:].rearrange("p (c tp i) -> p c tp i", c=4, tp=T)
            t01 = const_pool.tile([P, 4 * NG], FP32, tag="t01")
            t23 = const_pool.tile([P, 4 * NG], FP32, tag="t23")
            kvs = const_pool.tile([P, 4 * NG], FP32, tag="kvs")
            t01v = t01[:].rearrange("p (c i) -> p c i", c=4)
            t23v = t23[:].rearrange("p (c i) -> p c i", c=4)
            nc.vector.tensor_tensor(t01v, kv4[:, :, 0, :], kv4[:, :, 1, :], Alu.add)
            nc.vector.tensor_tensor(t23v, kv4[:, :, 2, :], kv4[:, :, 3, :], Alu.add)
            nc.vector.tensor_tensor(
                kvs[:].rearrange("p (c i) -> p c i", c=4), t01v, t23v, Alu.add
            )
            kvs_v = kvs[:].rearrange("p (c i) -> p c i", c=4)
            wg_v = wgT_sb[:].rearrange("p (c o) -> p c o", c=4)

            # gT[o, i] = sum_d wgT[d, o] * kvsum[d, i]: output d' on partitions
            # so the gate is a per-partition scalar for the d-major main loop.
            gsb = const_pool.tile([P, DC * NG], FP32, tag="gsb")
            for mc in range(DC):
                psum_g = psg_pool.tile([P, NG], FP32, tag=f"psg{mc}",
                                       name=f"psum_g{mc}")
                for kc in range(4):
                    nc.tensor.matmul(
                        psum_g[:], wg_v[:, kc, mc * P:(mc + 1) * P],
                        kvs_v[:, kc, :], start=(kc == 0), stop=(kc == 3),
                    )
                nc.vector.tensor_copy(gsb[:, mc * NG:(mc + 1) * NG], psum_g[:])

            if timing_mode:
                # fill internal q (values irrelevant for timing; reuse wgT_sb)
                for t in range(T):
                    for b in range(B_LOC):
                        nc.sync.dma_start(
                            q_v[t, b, :, 0:2, :],
                            wgT_sb[:].rearrange("p (c o) -> p c o", c=2))
                        nc.sync.dma_start(
                            q_v[t, b, :, 2:4, :],
                            wgT_sb[:].rearrange("p (c o) -> p c o", c=2))
                nc.sync.dma_start(dummy, wgT_sb[:, :16])  # satisfy external output

            import contextlib
            if timing_mode and repeats > 1:
                assert repeats % UNROLL == 0
                rep_ctx = tc.For_i(0, repeats // UNROLL, 1)
                inner_reps = UNROLL
            else:
                rep_ctx = contextlib.nullcontext()
                inner_reps = 1

            # queue plan (HWDGE transfers occupy the issuing engine, SWDGE
            # transfers run async off a ~1us Pool desc-gen): loads on SP,
            # stores mostly SWDGE/Pool, 4 on ACT (fits beside the sigmoids).
            store_eng = {}
            for b in range(B_LOC):
                for t in range(T):
                    store_eng[(b, t)] = nc.gpsimd
            for bt in ((0, 1), (1, 0), (2, 2), (3, 0)):
                store_eng[bt] = nc.scalar
            load_eng = {}
            for b in range(B_LOC):
                for t in range(T):
                    load_eng[(t, b)] = nc.sync

            def g_ptr(t, b, dc):
                col = dc * NG + b * NKV + t
                return gsb[:, col:col + 1]

            H = FREE // 2  # half-tile split for the drain-critical last tile

            # ---- main loop: b-outer, t-inner (recurrence chain per b) ----
            # The per-b DVE chain is serial (ts/stt/mask all on DVE) so DVE
            # runs back-to-back; q tiles die within their b-chain, keeping
            # SBUF pressure low enough for the SP queue to prefetch ahead.
            with rep_ctx:
             for _inner in range(inner_reps):
              for b in range(B_LOC):
                w_prev = None
                for t in range(T):
                    qt = q_pool.tile([P, FREE], FP32, tag="q", name=f"q_{t}_{b}")
                    load_eng[(t, b)].dma_start(
                        qt[:].rearrange("p (dc n) -> p dc n", dc=DC), q_v[t, b])
                    last = (t == T - 1 and b == B_LOC - 1)
                    if t == 0:
                        # gate-multiply: b_0 = q * G  (2x-rate ts)
                        for dc in range(DC):
                            sl = qt[:, dc * NQ:(dc + 1) * NQ]
                            nc.vector.tensor_scalar(
                                sl, sl, g_ptr(t, b, dc), None, Alu.mult)
                    else:
                        # fused charge: b_t = (q * G) + W_{t-1}
                        for dc in range(DC):
                            sl = qt[:, dc * NQ:(dc + 1) * NQ]
                            nc.vector.scalar_tensor_tensor(
                                sl, sl, g_ptr(t, b, dc),
                                w_prev[:, dc * NQ:(dc + 1) * NQ],
                                Alu.mult, Alu.add,
                            )
                    # s = (b_t >= thr_t) as exact 0.0/1.0: sigmoid saturates
                    # for |x| > ~17 and the ACT affine is a true fma, so the
                    # sign of BIG*(b - thr) is exact.
                    st = s_pool.tile([P, FREE], BF16, tag="s", name=f"s_{t}_{b}")
                    o_v = st[:].rearrange("p (dc n) -> p dc n", dc=DC)
                    if not last:
                        nc.scalar.activation(
                            st[:], qt[:], mybir.ActivationFunctionType.Sigmoid,
                            bias=thr_bias[:, t:t + 1], scale=_BIG,
                        )
                        store_eng[(b, t)].dma_start(out_v[t, b], o_v)
                    else:
                        for h in range(2):
                            nc.scalar.activation(
                                st[:, h * H:(h + 1) * H],
                                qt[:, h * H:(h + 1) * H],
                                mybir.ActivationFunctionType.Sigmoid,
                                bias=thr_bias[:, t:t + 1], scale=_BIG,
                            )
                            eng = nc.sync if h == 0 else nc.scalar
                            eng.dma_start(
                                out_v[t, b, :, 2 * h:2 * h + 2, :],
                                st[:, h * H:(h + 1) * H].rearrange(
                                    "p (dc n) -> p dc n", dc=2),
                            )
                    if t < T - 1:
                        wt = w_pool.tile([P, FREE], FP32, tag="w",
                                         name=f"w_{t}_{b}")
                        nc.vector.scalar_tensor_tensor(
                            wt[:], qt[:], THR[t], qt[:], Alu.is_lt, Alu.mult
                        )
                        w_prev = wt
    nc.compile()
    return nc


_CACHED_NC = None


def _make_in_maps(q, kv, Wg):
    q = np.ascontiguousarray(q, dtype=np.float32)
    kv = np.asarray(kv, dtype=np.float32)
    Wg = np.ascontiguousarray(Wg, dtype=np.float32)

    # transposed so the contraction dim lands on partitions
    wgT = np.ascontiguousarray(Wg.T)

    # fold the gate mean (1/T) and the 2^(t-1) LIF rescaling into kv: the
    # gate used at step t is nkv == t.
    fac = (2.0 ** (np.arange(NKV) - 1)).astype(np.float32) / np.float32(T)
    kv_s = kv * fac[None, None, :, None]

    # partition-interleaved d-major q for the device: [T, B, P, DC, NQ]
    # with row (p, dc) holding q[..., :, dc*128+p]
    qT = np.ascontiguousarray(
        q.reshape(T, B, NQ, DC, P).transpose(0, 1, 4, 3, 2)
    ).reshape(T, B, D, NQ)

    in_maps = []
    for i in range(N_CORES):
        b0 = i * B_LOC
        q_i = np.ascontiguousarray(qT[:, b0:b0 + B_LOC])
        kv_i = kv_s[:, b0:b0 + B_LOC]  # [T, B_LOC, NKV, D]
        kvT_i = np.ascontiguousarray(
            kv_i.transpose(3, 0, 1, 2).reshape(D, T * B_LOC * NKV)
        )
        in_maps.append({"q": q_i, "kvT": kvT_i, "wgT": wgT})
    return in_maps


def kernel(q: np.ndarray, kv: np.ndarray, Wg: np.ndarray) -> np.ndarray:
    global _CACHED_NC
    if _CACHED_NC is None:
        _CACHED_NC = build_kernel()
    nc = _CACHED_NC

    in_maps = _make_in_maps(q, kv, Wg)
    res = run_bass_kernel_spmd(nc, in_maps, core_ids=list(range(N_CORES)))
    # device out is [T, B_LOC, P, DC, NQ] (p-interleaved d-major); invert to
    # [T, B, NQ, D] with d = dc*128 + p
    out = np.concatenate([np.asarray(r["out"]) for r in res.results], axis=1)
    out = out.reshape(T, B, P, DC, NQ).transpose(0, 1, 4, 3, 2)
    return np.ascontiguousarray(out.reshape(T, B, NQ, D), dtype=np.float32)


if __name__ == "__main__":
    rng = np.random.default_rng(0)
    q = rng.standard_normal((T, B, NQ, D), dtype=np.float32)
    kv = rng.standard_normal((T, B, NKV, D), dtype=np.float32)
    Wg = (rng.standard_normal((D, D), dtype=np.float32) / np.sqrt(D)).astype(np.float32)
    o = kernel(q, kv, Wg)
    print("out", o.shape, o.dtype, "mean", o.mean())


# revision 20
# speedup vs baseline: 1.2455x; 1.2455x over previous
"""Trainium2 Bass kernel for nn_MemoryUpdate (gated LIF memory update).

Reference computation (fp32):
    k         = einsum('tbnd,od->tbno', kv, Wg)          # kv @ Wg^T
    gate_mean = mean_t'( k[t', b, nkv, d] )              # [Nkv, B, 1, D], Nkv == T
    update    = gate_mean[t, b, d] * q[t, b, n, d]       # broadcast over n
    spikes    = LIF over t: v' = (v + u)/2 ; s = v' >= 0.5 ; v = v' * (1 - s)

Shapes: q [4, 32, 1024, 512], kv [4, 32, 4, 512], Wg [512, 512] -> out [4, 32, 1024, 512].

Strategy: data-parallel over B across 8 cores (B_loc = 4), d-major on device.

The LIF recurrence is rescaled by 2^t so the 1/2 leak becomes exact integer
powers folded into per-step constants:  b_t := 2^t v'_t satisfies
    b_t = G_t (x) q_t + W_{t-1},   G_t = 2^{t-1} gate_mean_t   (host-folded)
    s_t = (b_t >= thr_t),          thr_t = 2^{t-1}
    W_t = (b_t < thr_t) * b_t
Host feeds q TRANSPOSED to [T, B, D, Nq] so d lands on SBUF partitions and the
gate G_t[d] becomes a per-partition scalar: the charge step is then ONE DVE
scalar_tensor_tensor (b = (q * g_ptr) + W) instead of a broadcast-multiply plus
add, and t=0 is a half-rate tensor_scalar. Per-core DVE drops ~174us -> ~116us.

Per core per (t, b): tile [128 part = d%128, free = (d//128, n)] (4 KiB
contiguous DRAM runs per (dc, n)).  Engine budget (model, per core):
  DVE  ~116us: charge stt x48 slices + t0 ts x16 + reset-mask stt x12
  ACT  ~62us:  threshold sigmoid (saturating, exact 0/1) x16 + 7 store issues
  SP   queue:  16 q loads (+kvT/wgT) ~97us
  ACT  queue:  7 stores ~41us;  Pool queue: 7 stores ~41us; 2 stores on SP
  PE:   tiny gate matmul gT[d, (b,t)] = sum_d' WgT[d', d] kvsum[d', (b,t)]
"""

import sys

for p in ("/opt/trn_rl_repo", "/root/.axon_site/_ro/trn_rl_repo"):
    if p not in sys.path:
        sys.path.insert(0, p)

import numpy as np

import concourse.bass as bass
import concourse.mybir as mybir
import concourse.tile as tile
from concourse import bacc
from concourse.bass_utils import run_bass_kernel_spmd

# Problem constants (hardcoded per harness contract)
T, B, NQ, NKV, D = 4, 32, 1024, 4, 512
N_CORES = 8
B_LOC = B // N_CORES  # 4
V_TH = 0.5
P = 128               # partitions
DC = D // P           # 4 d-chunks
FREE = DC * NQ        # 4096 free elements per tile

FP32 = mybir.dt.float32
OUT_DT = mybir.dt.uint8
Alu = mybir.AluOpType
UNROLL = 8  # static inner unroll inside the timing-mode For_i loop
_BIG = 1.0e30  # threshold-comparison scale; saturates sigmoid to exact 0/1
THR = [0.5 * (2.0 ** t) for t in range(T)]  # per-step threshold 2^(t-1)


def build_kernel(repeats=1, timing_mode=False, num_devices=N_CORES,
                 use_accum=False):
    # Bacc (not raw Bass): its compile() legalizes multi-sem waits, which the
    # walrus CoreV3 codegen can't carry on a single compute instruction.
    nc = bacc.Bacc("TRN2", target_bir_lowering=False, debug=False,
                   num_devices=num_devices)

    if timing_mode:
        # timing-only variant: big tensors live in internal DRAM so the wall
        # clock isn't dominated by host<->device transfers; the main body runs
        # `repeats` times in an on-device loop.
        q = nc.dram_tensor("q_int", [T, B_LOC, D, NQ], FP32).ap()
        out = nc.dram_tensor("out_int", [T, B_LOC, D, NQ], OUT_DT).ap()
        dummy = nc.dram_tensor("tiny_out", [P, 16], FP32, kind="ExternalOutput").ap()
    else:
        q = nc.dram_tensor("q", [T, B_LOC, D, NQ], FP32, kind="ExternalInput").ap()
        # spikes are exactly 0.0/1.0 -> uint8 is lossless and quarters the
        # store traffic; the host casts back to fp32
        out = nc.dram_tensor("out", [T, B_LOC, D, NQ], OUT_DT, kind="ExternalOutput").ap()
        dummy = None
    kvT = nc.dram_tensor("kvT", [D, T * B_LOC * NKV], FP32, kind="ExternalInput").ap()
    wgT = nc.dram_tensor("wgT", [D, D], FP32, kind="ExternalInput").ap()

    # Host supplies q/out in partition-interleaved d-major layout
    # [t, b, p, dc, n] (row p*DC+dc holds d = dc*128+p), so each partition's
    # (dc, n) free block is 16 KiB contiguous in DRAM -> fat DMA descriptors.
    q_v = q.rearrange("t b (p dc) n -> t b p dc n", p=P, dc=DC)
    out_v = out.rearrange("t b (p dc) n -> t b p dc n", p=P, dc=DC)
    # kvT rows: d = c*128 + p ; cols: i = t'*16 + b*4 + nkv
    kvT_v = kvT.rearrange("(c p) i -> p c i", p=P)
    wgT_v = wgT.rearrange("(c p) o -> p c o", p=P)
    NI = T * B_LOC * NKV  # 64
    NG = B_LOC * NKV      # 16 gate columns (b*4 + t)

    with tile.TileContext(nc) as tc:
        with (
            tc.tile_pool(name="const", bufs=1) as const_pool,
            tc.tile_pool(name="qp", bufs=5) as q_pool,
            tc.tile_pool(name="wp", bufs=2) as w_pool,
            tc.tile_pool(name="sp", bufs=3) as s_pool,
            tc.tile_pool(name="psg", bufs=1, space="PSUM") as psg_pool,
        ):
            # per-partition bias vectors for the threshold sigmoid (one per t);
            # also feeds a dummy activation that pre-loads the sigmoid ACT
            # table so the first real threshold doesn't pay the 1.3us load.
            thr_bias = const_pool.tile([P, T], FP32, tag="thrb")
            for t in range(T):
                nc.vector.memset(thr_bias[:, t:t + 1], -THR[t] * _BIG)
            # ---- gate computation (kvT/wgT ride the ACT queue so the SP
            # queue starts streaming q immediately) ----
            kvT_sb = const_pool.tile([P, 4 * NI], FP32, tag="kvT")
            nc.scalar.dma_start(kvT_sb[:].rearrange("p (c i) -> p c i", c=4), kvT_v)
            wgT_sb = const_pool.tile([P, 4 * D], FP32, tag="wgT")
            nc.scalar.dma_start(wgT_sb[:].rearrange("p (c o) -> p c o", c=4), wgT_v)
            # dummy activation pre-loads the sigmoid ACT table (~1.3us)
            warm = const_pool.tile([P, 1], FP32, tag="warm")
            nc.scalar.activation(
                warm[:], thr_bias[:, 0:1],
                mybir.ActivationFunctionType.Sigmoid, bias=0.0, scale=0.0,
            )

            # sum over t' of kvT (free layout per chunk: i = t'*16 + (b*4+nkv));
            # the 2^(t-1)/T gate scaling is folded into kvT host-side.
            kv4 = kvT_sb[:].rearrange("p (c tp i) -> p c tp i", c=4, tp=T)
            t01 = const_pool.tile([P, 4 * NG], FP32, tag="t01")
            t23 = const_pool.tile([P, 4 * NG], FP32, tag="t23")
            kvs = const_pool.tile([P, 4 * NG], FP32, tag="kvs")
            t01v = t01[:].rearrange("p (c i) -> p c i", c=4)
            t23v = t23[:].rearrange("p (c i) -> p c i", c=4)
            nc.vector.tensor_tensor(t01v, kv4[:, :, 0, :], kv4[:, :, 1, :], Alu.add)
            nc.vector.tensor_tensor(t23v, kv4[:, :, 2, :], kv4[:, :, 3, :], Alu.add)
            nc.vector.tensor_tensor(
                kvs[:].rearrange("p (c i) -> p c i", c=4), t01v, t23v, Alu.add
            )
            kvs_v = kvs[:].rearrange("p (c i) -> p c i", c=4)
            wg_v = wgT_sb[:].rearrange("p (c o) -> p c o", c=4)

            # gT[o, i] = sum_d wgT[d, o] * kvsum[d, i]: output d' on partitions
            # so the gate is a per-partition scalar for the d-major main loop.
            gsb = const_pool.tile([P, DC * NG], FP32, tag="gsb")
            for mc in range(DC):
                psum_g = psg_pool.tile([P, NG], FP32, tag=f"psg{mc}",
                                       name=f"psum_g{mc}")
                for kc in range(4):
                    nc.tensor.matmul(
                        psum_g[:], wg_v[:, kc, mc * P:(mc + 1) * P],
                        kvs_v[:, kc, :], start=(kc == 0), stop=(kc == 3),
                    )
                nc.vector.tensor_copy(gsb[:, mc * NG:(mc + 1) * NG], psum_g[:])

            if timing_mode:
                # fill internal q (values irrelevant for timing; reuse wgT_sb)
                for t in range(T):
                    for b in range(B_LOC):
                        nc.sync.dma_start(
                            q_v[t, b, :, 0:2, :],
                            wgT_sb[:].rearrange("p (c o) -> p c o", c=2))
                        nc.sync.dma_start(
                            q_v[t, b, :, 2:4, :],
                            wgT_sb[:].rearrange("p (c o) -> p c o", c=2))
                nc.sync.dma_start(dummy, wgT_sb[:, :16])  # satisfy external output

            import contextlib
            if timing_mode and repeats > 1:
                assert repeats % UNROLL == 0
                rep_ctx = tc.For_i(0, repeats // UNROLL, 1)
                inner_reps = UNROLL
            else:
                rep_ctx = contextlib.nullcontext()
                inner_reps = 1

            # queue plan (HWDGE transfers occupy the issuing engine, SWDGE
            # transfers run async off a ~1us Pool desc-gen): loads on SP,
            # stores mostly SWDGE/Pool, 4 on ACT (fits beside the sigmoids).
            store_eng = {}
            for b in range(B_LOC):
                for t in range(T):
                    store_eng[(b, t)] = nc.gpsimd
            for bt in ((0, 1), (1, 0), (2, 2), (3, 0)):
                store_eng[bt] = nc.scalar
            load_eng = {}
            for b in range(B_LOC):
                for t in range(T):
                    load_eng[(t, b)] = nc.sync

            def g_ptr(t, b, dc):
                col = dc * NG + b * NKV + t
                return gsb[:, col:col + 1]

            H = FREE // 2  # half-tile split for the drain-critical last tile

            # ---- main loop: b-outer, t-inner (recurrence chain per b) ----
            # The per-b DVE chain is serial (ts/stt/mask all on DVE) so DVE
            # runs back-to-back; q tiles die within their b-chain, keeping
            # SBUF pressure low enough for the SP queue to prefetch ahead.
            with rep_ctx:
             for _inner in range(inner_reps):
              for b in range(B_LOC):
                w_prev = None
                for t in range(T):
                    qt = q_pool.tile([P, FREE], FP32, tag="q", name=f"q_{t}_{b}")
                    load_eng[(t, b)].dma_start(
                        qt[:].rearrange("p (dc n) -> p dc n", dc=DC), q_v[t, b])
                    last = (t == T - 1 and b == B_LOC - 1)
                    if t == 0:
                        # gate-multiply: b_0 = q * G  (2x-rate ts)
                        for dc in range(DC):
                            sl = qt[:, dc * NQ:(dc + 1) * NQ]
                            nc.vector.tensor_scalar(
                                sl, sl, g_ptr(t, b, dc), None, Alu.mult)
                    else:
                        # fused charge: b_t = (q * G) + W_{t-1}
                        for dc in range(DC):
                            sl = qt[:, dc * NQ:(dc + 1) * NQ]
                            nc.vector.scalar_tensor_tensor(
                                sl, sl, g_ptr(t, b, dc),
                                w_prev[:, dc * NQ:(dc + 1) * NQ],
                                Alu.mult, Alu.add,
                            )
                    # s = (b_t >= thr_t) as exact 0.0/1.0: sigmoid saturates
                    # for |x| > ~17 and the ACT affine is a true fma, so the
                    # sign of BIG*(b - thr) is exact.
                    st = s_pool.tile([P, FREE], OUT_DT, tag="s", name=f"s_{t}_{b}")
                    o_v = st[:].rearrange("p (dc n) -> p dc n", dc=DC)
                    if not last:
                        nc.scalar.activation(
                            st[:], qt[:], mybir.ActivationFunctionType.Sigmoid,
                            bias=thr_bias[:, t:t + 1], scale=_BIG,
                        )
                        store_eng[(b, t)].dma_start(out_v[t, b], o_v)
                    else:
                        for h in range(2):
                            nc.scalar.activation(
                                st[:, h * H:(h + 1) * H],
                                qt[:, h * H:(h + 1) * H],
                                mybir.ActivationFunctionType.Sigmoid,
                                bias=thr_bias[:, t:t + 1], scale=_BIG,
                            )
                            eng = nc.sync if h == 0 else nc.scalar
                            eng.dma_start(
                                out_v[t, b, :, 2 * h:2 * h + 2, :],
                                st[:, h * H:(h + 1) * H].rearrange(
                                    "p (dc n) -> p dc n", dc=2),
                            )
                    if t < T - 1:
                        wt = w_pool.tile([P, FREE], FP32, tag="w",
                                         name=f"w_{t}_{b}")
                        nc.vector.scalar_tensor_tensor(
                            wt[:], qt[:], THR[t], qt[:], Alu.is_lt, Alu.mult
                        )
                        w_prev = wt
    nc.compile()
    return nc


_CACHED_NC = None


def _make_in_maps(q, kv, Wg):
    q = np.ascontiguousarray(q, dtype=np.float32)
    kv = np.asarray(kv, dtype=np.float32)
    Wg = np.ascontiguousarray(Wg, dtype=np.float32)

    # transposed so the contraction dim lands on partitions
    wgT = np.ascontiguousarray(Wg.T)

    # fold the gate mean (1/T) and the 2^(t-1) LIF rescaling into kv: the
    # gate used at step t is nkv == t.
    fac = (2.0 ** (np.arange(NKV) - 1)).astype(np.float32) / np.float32(T)
    kv_s = kv * fac[None, None, :, None]

    # partition-interleaved d-major q for the device: [T, B, P, DC, NQ]
    # with row (p, dc) holding q[..., :, dc*128+p]
    qT = np.ascontiguousarray(
        q.reshape(T, B, NQ, DC, P).transpose(0, 1, 4, 3, 2)
    ).reshape(T, B, D, NQ)

    in_maps = []
    for i in range(N_CORES):
        b0 = i * B_LOC
        q_i = np.ascontiguousarray(qT[:, b0:b0 + B_LOC])
        kv_i = kv_s[:, b0:b0 + B_LOC]  # [T, B_LOC, NKV, D]
        kvT_i = np.ascontiguousarray(
            kv_i.transpose(3, 0, 1, 2).reshape(D, T * B_LOC * NKV)
        )
        in_maps.append({"q": q_i, "kvT": kvT_i, "wgT": wgT})
    return in_maps


def kernel(q: np.ndarray, kv: np.ndarray, Wg: np.ndarray) -> np.ndarray:
    global _CACHED_NC
    if _CACHED_NC is None:
        _CACHED_NC = build_kernel()
    nc = _CACHED_NC

    in_maps = _make_in_maps(q, kv, Wg)
    res = run_bass_kernel_spmd(nc, in_maps, core_ids=list(range(N_CORES)))
    # device out is [T, B_LOC, P, DC, NQ] (p-interleaved d-major); invert to
    # [T, B, NQ, D] with d = dc*128 + p
    out = np.concatenate([np.asarray(r["out"]) for r in res.results], axis=1)
    out = out.reshape(T, B, P, DC, NQ).transpose(0, 1, 4, 3, 2)
    return np.ascontiguousarray(out.reshape(T, B, NQ, D), dtype=np.float32)


if __name__ == "__main__":
    rng = np.random.default_rng(0)
    q = rng.standard_normal((T, B, NQ, D), dtype=np.float32)
    kv = rng.standard_normal((T, B, NKV, D), dtype=np.float32)
    Wg = (rng.standard_normal((D, D), dtype=np.float32) / np.sqrt(D)).astype(np.float32)
    o = kernel(q, kv, Wg)
    print("out", o.shape, o.dtype, "mean", o.mean())
